# revision 55
# baseline (speedup 1.0000x reference)
"""BitNet attention (B=2, S=2048, HID=2560, NH=20, NKV=5, HD=128, GQA=4) on 8 TRN2 cores.

Sharding: 2-way batch x 4-way head-group tensor parallel.
Core (b, g) computes q-heads [4g, 4g+1, 4g+2, 4g+3, 16+g] and kv-heads [g, 4]
for batch b (uniform q-head -> kv mapping across cores so one SPMD NEFF works:
local heads 0-3 -> kv slot 0, local head 4 -> kv slot 1).

Per-core device pipeline, fused per 512-wide seq block j (causal => attention
for block j only needs K/V of blocks <= j):
  - Q^T/K^T = W@X^T and V = X@Wv^T as per-head/per-chunk "units" (one PSUM
    bank + 20 accumulating matmuls each); evac + RoPE on DVE. Units for
    block j+1 are injected at head boundaries of block j's attention, which
    is ACT(exp)-bound, so the PE stays busy through it.
  - per head: S^T = K^T.T@Q^T, exp on ACT (scale=alpha); diagonal k-tiles
    run last in descending order with triangular q ranges (masked on DVE);
    AV in PSUM; softmax denominator = probs accumulated on DVE in 4
    independent chains (chained adds are latency-bound), then one
    ones-matmul + broadcast matmul on PE; per-head tail chains are
    software-pipelined one head late.
  - o-proj partials y^T = Wo'@(w * attn_out^T) in bf16.
Host: unpack ternary weights, build RoPE tables, sum partial y / sumsq over
the 4 cores of each batch, apply v/o scales and the RMSNorm per-seq scale
(per-seq scalars commute through the linear o-proj).
"""

import math
import numpy as np
import ml_dtypes
from contextlib import ExitStack

import concourse.bacc as bacc
import concourse.tile as tile
import concourse.mybir as mybir
from concourse import bass_utils

B, S, HID = 2, 2048, 2560
NH, NKV, HD = 20, 5, 128
THETA = 500000.0
RMS_EPS = 1e-6

N_CORES = 8
KT = HID // 128          # 20 k-tiles over hidden dim
J = S // 512             # 4 seq blocks of 512
SKT = S // 128           # 16 sk tiles
NQH = 5                  # q heads per core
NKVH = 2                 # kv heads per core

F32 = mybir.dt.float32
F32R = mybir.dt.float32r
BF16 = mybir.dt.bfloat16
F16 = mybir.dt.float16

_cache = {}

# schedule tuning knobs (sim-swept). Per block: "early" units are injected
# at the previous block's attention head boundaries; "late" units are
# injected inside the block's own attention at the given (head, pos).
TUNE = {"depth": 4, "bc_at": 3, "b1_at": 4, "c_at": 6,
        "plan": {
            1: {"early": ["q0", "k0", "k1", "q1", "q2", "q3"],
                "late": [("v0", (0, 0)), ("v1", (0, 2)), ("v2", (0, 4)),
                         ("v3", (0, 6)), ("q4", (1, 0))]},
            2: {"early": ["q0", "k0", "k1", "q1", "q2", "q3"],
                "late": [("v0", (0, 0)), ("v1", (0, 3)), ("v2", (0, 6)),
                         ("v3", (0, 8)), ("q4", (1, 0))]},
            3: {"early": ["q0", "k0", "k1"],
                "late": [("v0", (0, 0)), ("v1", (0, 3)), ("q1", (0, 6)),
                         ("v2", (0, 9)), ("v3", (0, 12)), ("q2", (1, 0)),
                         ("q3", (2, 0)), ("q4", (3, 0))]},
        }}


def _build(alpha: float, repeats: int):
    nc = bacc.Bacc("TRN2", target_bir_lowering=False, debug=False, num_devices=N_CORES)

    xt_d = nc.dram_tensor("xt", [HID, S], BF16, kind="ExternalInput")
    wq_d = nc.dram_tensor("wq", [HID, NQH * HD], BF16, kind="ExternalInput")
    wk_d = nc.dram_tensor("wk", [HID, NKVH * HD], BF16, kind="ExternalInput")
    wv_d = nc.dram_tensor("wv", [HID, NKVH * HD], BF16, kind="ExternalInput")
    wo_d = nc.dram_tensor("wo", [NQH * HD, HID], BF16, kind="ExternalInput")
    cos_d = nc.dram_tensor("cos", [HD, S], F16, kind="ExternalInput")
    sin_d = nc.dram_tensor("sin", [HD, S], F16, kind="ExternalInput")
    wn_d = nc.dram_tensor("wn", [HD, NQH], F32, kind="ExternalInput")
    dmask_d = nc.dram_tensor("dmask", [4, HD, 512], BF16, kind="ExternalInput")
    onc_d = nc.dram_tensor("onc", [HD, 1], F32R, kind="ExternalInput")
    onr_d = nc.dram_tensor("onr", [1, HD], F32R, kind="ExternalInput")
    y_d = nc.dram_tensor("y", [HID, S], F32, kind="ExternalOutput")
    ssq_d = nc.dram_tensor("ssq", [1, S], F32, kind="ExternalOutput")

    with tile.TileContext(nc) as tc, ExitStack() as octx:
        ps = octx.enter_context(tc.tile_pool(name="ps", bufs=6, space="PSUM"))
        kt_p = octx.enter_context(tc.tile_pool(name="ktp", bufs=1))
        v_p = octx.enter_context(tc.tile_pool(name="vp", bufs=1))
        qb_p = octx.enter_context(tc.tile_pool(name="qbp", bufs=12))
        const_p = octx.enter_context(tc.tile_pool(name="constp", bufs=1))
        w_p = octx.enter_context(tc.tile_pool(name="wp", bufs=1))
        xt_p = octx.enter_context(tc.tile_pool(name="xtp", bufs=2))
        rp_p = octx.enter_context(tc.tile_pool(name="rpp", bufs=4))
        pr_p = octx.enter_context(tc.tile_pool(name="prp", bufs=6))
        tw_p = octx.enter_context(tc.tile_pool(name="twp", bufs=7))
        mis_p = octx.enter_context(tc.tile_pool(name="misp", bufs=2))
        y_p = octx.enter_context(tc.tile_pool(name="yp", bufs=4))
        pa_p = octx.enter_context(tc.tile_pool(name="pap", bufs=3))

        def body(_it=None):
            # --- persistent SBUF for one iteration ---
            kt = kt_p.tile([128, NKVH * S], BF16, tag="kt", name="kt")
            vt = v_p.tile([128, SKT * NKVH * HD], BF16, tag="vt", name="vt")

            wq = w_p.tile([128, KT * NQH * HD], BF16, tag="wq", name="wq")
            wk = w_p.tile([128, KT * NKVH * HD], BF16, tag="wk", name="wk")
            wv = w_p.tile([128, KT * NKVH * HD], BF16, tag="wv", name="wv")
            wo = w_p.tile([128, NQH * HID], BF16, tag="wo", name="wo")

            def dma_w_chunk(dst, src_d, W, k0, k1):
                nc.sync.dma_start(
                    dst[:, k0 * W:k1 * W].rearrange("p (k o) -> p k o", k=k1 - k0),
                    src_d.ap()[k0 * 128:k1 * 128].rearrange("(k p) o -> p k o", p=128))

            def dma_xt_chunk(dst, j, k0, k1):
                nc.sync.dma_start(
                    dst[:, k0 * 512:k1 * 512].rearrange("p (k s) -> p k s", k=k1 - k0),
                    xt_d.ap()[k0 * 128:k1 * 128, j * 512:(j + 1) * 512]
                    .rearrange("(k p) s -> p k s", p=128))

            xts = [None] * J

            # prologue runs k/v units first, so stream xt+wk, then wv, wq
            xts[0] = xt_p.tile([128, KT * 512], BF16, tag="xt", name="xt0")
            for c in range(4):
                dma_xt_chunk(xts[0], 0, 5 * c, 5 * c + 5)
                dma_w_chunk(wk, wk_d, NKVH * HD, 5 * c, 5 * c + 5)
            for c in range(4):
                dma_w_chunk(wv, wv_d, NKVH * HD, 5 * c, 5 * c + 5)
            for c in range(4):
                dma_w_chunk(wq, wq_d, NQH * HD, 5 * c, 5 * c + 5)

            # constants / tables (needed slightly later than the first matmuls)
            cos_t = const_p.tile([HD, S], F16, tag="cos", name="cos")
            nc.sync.dma_start(cos_t[:], cos_d.ap())
            sin_t = const_p.tile([HD, S], F16, tag="sin", name="sin")
            nc.sync.dma_start(sin_t[:], sin_d.ap())
            onc = const_p.tile([HD, 1], F32R, tag="onc", name="onc")
            nc.sync.dma_start(onc[:], onc_d.ap())
            onr = const_p.tile([1, HD], F32R, tag="onr", name="onr")
            nc.sync.dma_start(onr[:], onr_d.ap())
            onc_bf = const_p.tile([HD, 1], BF16, tag="oncb", name="oncb")
            nc.any.memset(onc_bf[:], 1.0)
            # touch Exp early so the ACT table load hides under the prologue
            preheat = const_p.tile([1, 1], F32, tag="preheat", name="preheat")
            nc.scalar.activation(preheat[:], onc_bf[0:1, 0:1],
                                 mybir.ActivationFunctionType.Exp, scale=1.0)
            wn = const_p.tile([HD, NQH], F32, tag="wn", name="wn")
            nc.sync.dma_start(wn[:], wn_d.ap())
            dmask = const_p.tile([HD, 4 * 512], BF16, tag="dmask", name="dmask")
            for o in range(4):
                nc.sync.dma_start(dmask[:, o * 512:(o + 1) * 512], dmask_d.ap()[o])

            pendA = [None]   # (h, j, pacc0, av_ps, ssq_ps, tws)
            pendBC = [None]  # (h, j, drow, av_ps, ssq_ps, tws)
            pendB = [None]   # (h, j, dbc, av_ps, ssq_ps, tws)

            def emit_tail_a():
                # one ones-matmul over the DVE-accumulated probs sum gives
                # the softmax denominator row; evac to SBUF on ACT
                if pendA[0] is None:
                    return
                h, j, pacc0, av_ps, ssq_ps, tws = pendA[0]
                pendA[0] = None
                d_ps = ps.tile([1, 512], F32, tag="ps", name=f"pd{j}_{h}")
                nc.tensor.matmul(d_ps[:], onc_bf[:], pacc0[:],
                                 start=True, stop=True)
                drow = mis_p.tile([1, 512], F32R, tag="drow", name=f"dr{j}_{h}")
                nc.scalar.copy(drow[:], d_ps[:])
                pendBC[0] = (h, j, drow, av_ps, ssq_ps, tws)

            def emit_tail_bc():
                # broadcast the denominator row across partitions (PE)
                if pendBC[0] is None:
                    return
                h, j, drow, av_ps, ssq_ps, tws = pendBC[0]
                pendBC[0] = None
                dbc = ps.tile([128, 512], F32, tag="ps", name=f"db{j}_{h}")
                nc.tensor.matmul(dbc[:], onr[:], drow[:], start=True, stop=True)
                pendB[0] = (h, j, dbc, av_ps, ssq_ps, tws)

            pendC = [None]  # (h, j, tn, ssq_ps, tws)

            def emit_tail_b():
                # B1: normalize -> frees av_ps psum slot; DVE only
                if pendB[0] is None:
                    return
                h, j, dbc, av_ps, ssq_ps, tws = pendB[0]
                pendB[0] = None
                rec = mis_p.tile([128, 512], F32, tag="rec", name=f"rc{j}_{h}")
                nc.vector.reciprocal(rec[:], dbc[:])
                tn = mis_p.tile([128, 512], F32, tag="tn", name=f"tn{j}_{h}")
                nc.vector.tensor_mul(tn[:], av_ps[:], rec[:])
                pendC[0] = (h, j, tn, ssq_ps, tws)

            def emit_tail_c():
                # B2: sumsq matmul + norm-weight scale
                if pendC[0] is None:
                    return
                h, j, tn, ssq_ps, tws = pendC[0]
                pendC[0] = None
                sqt = mis_p.tile([128, 512], F32R, tag="sqt", name=f"sq{j}_{h}")
                nc.vector.tensor_mul(sqt[:], tn[:], tn[:])
                nc.tensor.matmul(ssq_ps[:], onc[:], sqt[:],
                                 start=(h == 0), stop=(h == NQH - 1))
                tw = tw_p.tile([128, 512], BF16, tag="tw", name=f"tw{j}_{h}")
                nc.vector.tensor_scalar_mul(tw[:], tn[:], wn[:, h:h + 1])
                tws.append(tw)

            qbs_all = [[None] * NQH for _ in range(J)]

            def rope_math2(dst, qr, j):
                sq = slice(j * 512, (j + 1) * 512)
                nc.vector.tensor_mul(dst, dst, cos_t[:, sq])
                nc.vector.tensor_mul(qr[:], qr[:], sin_t[:, sq])
                nc.vector.tensor_add(dst, dst, qr[:])

            def cp(dst, src, on_act):
                if on_act:
                    nc.scalar.copy(dst, src)
                else:
                    nc.vector.tensor_copy(dst, src)

            def rot_evac(psrc, nm, on_act):
                # rotate-half via two cross-partition copies (PSUM->SBUF);
                # ACT when it's idle (prologue/drain), DVE during attention
                qr = rp_p.tile([128, 512], F16, tag="trot", name=nm)
                cp(qr[0:64, :], psrc[64:128, :], on_act)
                cp(qr[64:128, :], psrc[0:64, :], on_act)
                return qr

            def make_units(j):
                # projection work units for block j: 20 accumulating matmuls
                # into one PSUM bank (tag psp) + evac/RoPE each. Units take
                # on_act=True when emitted in an ACT-idle phase.
                xt = xts[j]

                def qunit(m):
                    def emit(on_act=False):
                        psq = ps.tile([128, 512], F32, tag="psp", bufs=2,
                                      name=f"upq{j}_{m}")
                        for k in range(KT):
                            nc.tensor.matmul(
                                psq[:],
                                wq[:, k * 640 + m * 128: k * 640 + (m + 1) * 128],
                                xt[:, k * 512:(k + 1) * 512],
                                start=(k == 0), stop=(k == KT - 1))
                        qb = qb_p.tile([128, 512], BF16, tag="qb",
                                       name=f"qb{j}_{m}")
                        cp(qb[:], psq[:], on_act)
                        qr = rot_evac(psq, f"trq{j}_{m}", on_act)
                        qbs_all[j][m] = qb
                        rope_math2(qb[:], qr, j)
                    return emit

                def kunit(m):
                    def emit(on_act=False):
                        psk = ps.tile([128, 512], F32, tag="psp", bufs=2,
                                      name=f"upk{j}_{m}")
                        for k in range(KT):
                            nc.tensor.matmul(
                                psk[:],
                                wk[:, k * 256 + m * 128: k * 256 + (m + 1) * 128],
                                xt[:, k * 512:(k + 1) * 512],
                                start=(k == 0), stop=(k == KT - 1))
                        kd = kt[:, m * S + j * 512: m * S + (j + 1) * 512]
                        cp(kd, psk[:], on_act)
                        qr = rot_evac(psk, f"trk{j}_{m}", on_act)
                        rope_math2(kd, qr, j)
                    return emit

                def vunit(t):
                    def emit(on_act=False):
                        psv = ps.tile([128, NKVH * HD], F32, tag="psp", bufs=2,
                                      name=f"upv{j}_{t}")
                        for k in range(KT):
                            nc.tensor.matmul(
                                psv[:],
                                xt[:, k * 512 + t * 128: k * 512 + (t + 1) * 128],
                                wv[:, k * 256:(k + 1) * 256],
                                start=(k == 0), stop=(k == KT - 1))
                        i = 4 * j + t
                        cp(vt[:, i * 256:(i + 1) * 256], psv[:], on_act)
                    return emit

                units = {f"q{m}": qunit(m) for m in range(NQH)}
                units.update({f"k{m}": kunit(m) for m in range(NKVH)})
                units.update({f"v{t}": vunit(t) for t in range(4)})
                return units

            def prefetch_xt(j):
                if j < J and xts[j] is None:
                    xts[j] = xt_p.tile([128, KT * 512], BF16, tag="xt",
                                       name=f"xt{j}")
                    for c in range(4):
                        dma_xt_chunk(xts[j], j, 5 * c, 5 * c + 5)

            # prologue: block 0's projection units back-to-back; k/v first
            # (their weights stream in ahead of the larger wq)
            units0 = make_units(0)
            for nm in ["k0", "k1", "v0", "v1", "v2", "v3",
                       "q0", "q1", "q2", "q3", "q4"]:
                units0[nm](on_act=True)

            late_units = {}
            for j in range(J):
                sq = slice(j * 512, (j + 1) * 512)
                prefetch_xt(j + 1)
                if j + 1 < J:
                    nxt = make_units(j + 1)
                    plan = TUNE["plan"][j + 1]
                    pending_units = [nxt[nm] for nm in plan["early"]]
                    late_units[j + 1] = [(nxt[nm], pt) for nm, pt in plan["late"]]
                else:
                    pending_units = []
                n_early = len(pending_units)
                late_pts = {}
                for u, pt in late_units.pop(j, []):
                    assert pt[0] < NQH and pt[1] < 4 * j + 4, "late pt OOB"
                    late_pts.setdefault(pt, []).append(u)
                if j == 0:
                    # wo needed only at the first o-proj; start its DMA now
                    nc.sync.dma_start(
                        wo[:].rearrange("p (h o) -> p h o", h=NQH),
                        wo_d.ap().rearrange("(h p) o -> p h o", p=128))

                # ---- attention ----
                # Tile schedule: off-diagonal k-tiles ascending (full q
                # width), then diagonal k-tiles in descending offset order
                # with a triangular q range [128*o, 512). The first tile
                # processed is always full-width (off-diag 0, or diag o=0
                # for j=0) and owns the PSUM start flag; the stop flag goes
                # on the last processed tile (hardware ignores it).
                ni = 4 * j + 4
                if j == 0:
                    sched = [(0, 0)] + [(o, 128 * o) for o in (3, 2, 1)]
                else:
                    sched = [(i, 0) for i in range(4 * j)] + \
                            [(4 * j + o, 128 * o) for o in (3, 2, 1, 0)]
                ssq_ps = ps.tile([1, 512], F32, tag="ps", name=f"pss{j}")
                tws = []
                for h in range(NQH):
                    kvl = 0 if h < 4 else 1
                    qr = qbs_all[j][h][:]
                    av_ps = ps.tile([128, 512], F32, tag="ps", name=f"pav{j}_{h}")
                    # probs accumulate on DVE in 4 independent chains (chained
                    # adds are latency-bound on the DVE, ~2.7x slower than
                    # throughput; 4 chains hide it), combined at head end
                    pacc = [pa_p.tile([128, 512], F16, tag="pacc", bufs=9,
                                      name=f"pa{j}_{h}_{c}") for c in range(4)]
                    queue = []

                    def flush_one():
                        pp, pi, q0, pos = queue.pop(0)
                        nc.tensor.matmul(
                            av_ps[:, q0:512],
                            vt[:, pi * 256 + kvl * 128: pi * 256 + kvl * 128 + 128],
                            pp, start=(pos == 0), stop=(pos == ni - 1),
                            skip_group_check=(j == 0))

                    bc_at = min(TUNE["bc_at"], ni - 2)
                    b1_at = min(TUNE["b1_at"], ni - 1)
                    dadds = []
                    for pos, (i, q0) in enumerate(sched):
                        for u in late_pts.get((h, pos), ()):
                            u()
                        s_ps = ps.tile([128, 512], F32, tag="ps", name=f"pS{j}_{h}_{i}")
                        nc.tensor.matmul(
                            s_ps[:, q0:512],
                            kt[:, kvl * S + i * 128: kvl * S + (i + 1) * 128],
                            qr[:, q0:512], start=True, stop=True)
                        if len(queue) >= TUNE["depth"]:
                            flush_one()
                        probs = pr_p.tile([128, 512], BF16, tag="probs",
                                          name=f"pr{j}_{h}_{i}")
                        pv = probs[:, q0:512]
                        nc.scalar.activation(
                            pv, s_ps[:, q0:512],
                            mybir.ActivationFunctionType.Exp, scale=alpha)
                        def pacc_op(pv_, q0_, pos_):
                            c = pos_ % 4
                            if pos_ < 4:
                                nc.vector.tensor_copy(pacc[c][:, q0_:512], pv_)
                            else:
                                nc.vector.tensor_add(pacc[c][:, q0_:512],
                                                     pacc[c][:, q0_:512], pv_)
                        if i >= 4 * j:
                            o = i - 4 * j
                            nc.vector.tensor_mul(
                                pv, pv, dmask[:, o * 512 + q0:(o + 1) * 512])
                            # diagonal pacc adds deferred past the AV drain so
                            # masks aren't queued behind them on DVE
                            dadds.append((pv, q0, pos))
                        else:
                            pacc_op(pv, q0, pos)
                        queue.append((pv, i, q0, pos))
                        if pos == 1:
                            emit_tail_a()   # denominator ones-matmul + drow
                        if pos == bc_at:
                            emit_tail_bc()  # broadcast matmul (PE)
                        if pos == b1_at:
                            emit_tail_b()   # recip+normalize: frees av slot
                        if pos == TUNE["c_at"]:
                            emit_tail_c()   # sumsq + tw
                    while queue:
                        flush_one()
                    for pv, q0, pos in dadds:
                        pacc_op(pv, q0, pos)
                    # combine the 4 chains into pacc[0]; chain c's valid
                    # width is that of its first tile (sched[c])
                    for c in (1, 2, 3):
                        if c < ni:
                            cq0 = sched[c][1]
                            nc.vector.tensor_add(pacc[0][:, cq0:512],
                                                 pacc[0][:, cq0:512],
                                                 pacc[c][:, cq0:512])
                    emit_tail_bc()  # short blocks: flush pending stages
                    emit_tail_c()
                    pendA[0] = (h, j, pacc[0], av_ps, ssq_ps, tws)
                    # inject next block's projection units: attention is
                    # ACT(exp)-bound, these keep the PE busy
                    nb = (n_early + NQH - 1 - h) // NQH  # even spread over heads
                    for _ in range(nb):
                        if pending_units:
                            pending_units.pop(0)()

                while pending_units:
                    pending_units.pop(0)(on_act=True)

                srow = mis_p.tile([1, 512], F32, tag="srow", name=f"sr{j}")
                emit_tail_a()  # last head's denominator matmul + drow

                # ---- o-proj; last head's tail overlapped inside first chunk
                # (2-wide chunks: the 6-buf ps ring can't hold 4 y tiles
                # alongside ssq/av/dbc) ----
                chunks = [(2 * c, 2 * c + 2) for c in range(10)]
                first = True
                for (m0, m1) in chunks:
                    y_pss = [ps.tile([128, 512], F32, tag="ps", name=f"py{j}_{m}")
                             for m in range(m0, m1)]
                    for h in range(NQH):
                        if first and h == 1:
                            emit_tail_bc()
                        if first and h == 2:
                            emit_tail_b()
                        if first and h == 3:
                            emit_tail_c()
                            nc.scalar.copy(srow[:], ssq_ps[:])
                            nc.sync.dma_start(ssq_d.ap()[:, sq], srow[:])
                            first = False
                        for mi, m in enumerate(range(m0, m1)):
                            nc.tensor.matmul(
                                y_pss[mi][:],
                                wo[:, h * HID + m * 128: h * HID + (m + 1) * 128],
                                tws[h][:], start=(h == 0), stop=(h == NQH - 1))
                    for mi, m in enumerate(range(m0, m1)):
                        ysb = y_p.tile([128, 512], F32, tag="ysb", name=f"y{j}_{m}")
                        if m % 2 == 0:
                            nc.scalar.copy(ysb[:], y_pss[mi][:])
                        else:
                            nc.vector.tensor_copy(ysb[:], y_pss[mi][:])
                        nc.sync.dma_start(
                            y_d.ap()[m * 128:(m + 1) * 128, sq], ysb[:])

        if repeats > 1:
            with tc.For_i(0, repeats) as _i:
                body(_i)
        else:
            body()

    nc.compile()
    return nc


def _unpack_ternary(packed: np.ndarray) -> np.ndarray:
    M, Kp = packed.shape
    nb = Kp // 32
    b = packed.reshape(M, nb, 32)
    f = np.stack([(b >> 6) & 3, (b >> 4) & 3, (b >> 2) & 3, b & 3], axis=2)
    return f.reshape(M, nb * 128).astype(np.float32) - 1.0


def _rope_tables():
    inv = 1.0 / (THETA ** (np.arange(0, HD, 2, dtype=np.float64) / HD))  # (64,)
    t = np.arange(S, dtype=np.float64)
    fr = t[None, :] * inv[:, None]          # (64, S)
    cos = np.concatenate([np.cos(fr), np.cos(fr)], axis=0)      # (128, S)
    sin = np.concatenate([-np.sin(fr), np.sin(fr)], axis=0)     # signed
    return cos.astype(np.float16), sin.astype(np.float16)


def _diag_masks():
    m = np.zeros((4, HD, 512), dtype=ml_dtypes.bfloat16)
    q = np.arange(512)[None, :]
    p = np.arange(HD)[:, None]
    for o in range(4):
        m[o] = (q >= p + 128 * o).astype(ml_dtypes.bfloat16)
    return m


def make_in_maps(hidden_states, q_w, k_w, v_w, o_w, attn_norm_w):
    wq_f = _unpack_ternary(np.asarray(q_w))     # (2560, 2560)
    wk_f = _unpack_ternary(np.asarray(k_w))     # (640, 2560)
    wv_f = _unpack_ternary(np.asarray(v_w))     # (640, 2560)
    wo_f = _unpack_ternary(np.asarray(o_w))     # (2560, 2560) [out, in]
    cos, sin = _rope_tables()
    dmask = _diag_masks()
    onc = np.ones((HD, 1), np.float32)
    onr = np.ones((1, HD), np.float32)
    wnorm = np.asarray(attn_norm_w, np.float32)
    hs = np.asarray(hidden_states)

    bf = ml_dtypes.bfloat16
    in_maps = []
    for c in range(N_CORES):
        b, g = c // 4, c % 4
        qheads = [4 * g, 4 * g + 1, 4 * g + 2, 4 * g + 3, 16 + g]
        kvheads = [g, 4]
        qrows = np.concatenate([wq_f[h * HD:(h + 1) * HD] for h in qheads], 0)
        krows = np.concatenate([wk_f[h * HD:(h + 1) * HD] for h in kvheads], 0)
        vrows = np.concatenate([wv_f[h * HD:(h + 1) * HD] for h in kvheads], 0)
        ocols = np.concatenate([wo_f[:, h * HD:(h + 1) * HD] for h in qheads], 1)
        wn = np.stack([wnorm[h * HD:(h + 1) * HD] for h in qheads], 1)  # (128, 5)
        in_maps.append({
            "xt": np.ascontiguousarray(hs[b].T).astype(bf),
            "wq": np.ascontiguousarray(qrows.T).astype(bf),
            "wk": np.ascontiguousarray(krows.T).astype(bf),
            "wv": np.ascontiguousarray(vrows.T).astype(bf),
            "wo": np.ascontiguousarray(ocols.T).astype(bf),
            "cos": cos, "sin": sin,
            "wn": np.ascontiguousarray(wn),
            "dmask": dmask, "onc": onc, "onr": onr,
        })
    return in_maps


def postprocess(results, v_scale, o_scale):
    out = np.empty((B, S, HID), np.float32)
    for b in range(B):
        y = np.zeros((HID, S), np.float64)
        ss = np.zeros((S,), np.float64)
        for g in range(4):
            r = results[b * 4 + g]
            y += r["y"].astype(np.float64)
            ss += r["ssq"][0].astype(np.float64)
        var = ss * (float(v_scale) ** 2) / HID
        rms = 1.0 / np.sqrt(var + RMS_EPS)
        out[b] = (y.T * (rms[:, None] * float(v_scale) * float(o_scale))).astype(np.float32)
    return out


def _get_nc(alpha: float, repeats: int = 1):
    key = (round(alpha, 12), repeats)
    if key not in _cache:
        _cache[key] = _build(alpha, repeats)
    return _cache[key]


def kernel(hidden_states, attention_mask, q_w, k_w, v_w, o_w,
           q_scale, k_scale, v_scale, o_scale, attn_norm_w):
    alpha = float(q_scale) * float(k_scale) / math.sqrt(HD)
    nc = _get_nc(alpha, 1)
    in_maps = make_in_maps(hidden_states, q_w, k_w, v_w, o_w, attn_norm_w)
    res = bass_utils.run_bass_kernel_spmd(nc, in_maps, core_ids=list(range(N_CORES)))
    return postprocess(res.results, v_scale, o_scale)
